# revision 1
# baseline (speedup 1.0000x reference)
"""MoE (top-2 of 8 experts, D=768, FF=3072) on 8 Trainium2 NeuronCores.

Strategy: expert-parallel. The router (0.05 GFLOP) runs on host; tokens are
dispatched to their top-2 experts on host, each core runs one expert's FFN
over its routed tokens (the 77 GFLOP that matter), and the host applies the
softmax-weighted combine.

Device layout puts tokens on the matmul free axis, so both matmuls contract
naturally over the partition axis with zero on-device transposes:
    HT[f,t] = relu(sum_d W1[d,f] * XT[d,t] + b1[f])   lhsT=W1, rhs=XT
    YT[d,t] =      sum_f W2[f,d] * HT[f,t] + b2[d]    lhsT=W2, rhs=HT
Inputs are fp16 (well-scaled data; PSUM accumulates fp32), epilogues fp32.
"""

import numpy as np

import concourse.bass as bass
import concourse.tile as tile
from concourse import bacc, mybir
from concourse import bass_utils

D_MODEL = 768
N_EXPERTS = 8
TOP_K = 2
D_FF = 3072
P = 128
KO = D_MODEL // P     # 6   contraction chunks for MM1 / output tiles for MM2
FO = D_FF // P        # 24  output tiles for MM1 / contraction chunks for MM2
CHUNK = 512           # token chunk (PSUM free dim, one fp32 bank)

_program_cache: dict[int, object] = {}


def _token_chunks(C):
    chunks = []
    t = 0
    while t < C:
        n = min(CHUNK, C - t)
        chunks.append((t, n))
        t += n
    return chunks


def _build_program(C):
    """Bass program for one expert's FFN over C routed tokens (SPMD x8)."""
    if C in _program_cache:
        return _program_cache[C]

    fp16 = mybir.dt.float16
    fp32 = mybir.dt.float32
    nc = bacc.Bacc("TRN2", target_bir_lowering=False, debug=False,
                   enable_asserts=True, num_devices=N_EXPERTS)

    xt_d = nc.dram_tensor("xt", [P, KO, C], fp16, kind="ExternalInput").ap()
    w1_d = nc.dram_tensor("w1", [P, KO, D_FF], fp16, kind="ExternalInput").ap()
    w2_d = nc.dram_tensor("w2", [P, FO, D_MODEL], fp16, kind="ExternalInput").ap()
    b1_d = nc.dram_tensor("b1c", [P, FO], fp32, kind="ExternalInput").ap()
    b2_d = nc.dram_tensor("b2c", [P, KO], fp32, kind="ExternalInput").ap()
    yt_d = nc.dram_tensor("yt", [P, KO, C], fp32, kind="ExternalOutput").ap()

    with tile.TileContext(nc) as tc:
        with (
            tc.tile_pool(name="wpool", bufs=1) as wpool,
            tc.tile_pool(name="hpool", bufs=2) as hpool,
            tc.tile_pool(name="ypool", bufs=2) as ypool,
            tc.tile_pool(name="pspool", bufs=4, space="PSUM") as pspool,
        ):
            w1_sb = wpool.tile([P, KO, D_FF], fp16)
            xt_sb = wpool.tile([P, KO, C], fp16)
            b1_sb = wpool.tile([P, FO], fp32)
            w2_sb = wpool.tile([P, FO, D_MODEL], fp16)
            b2_sb = wpool.tile([P, KO], fp32)
            # order matters: first matmul needs w1+xt(+b1); w2 streams in
            # under MM1 of the first chunk.
            nc.sync.dma_start(w1_sb[:], w1_d[:])
            nc.sync.dma_start(xt_sb[:], xt_d[:])
            nc.sync.dma_start(b1_sb[:], b1_d[:])
            nc.sync.dma_start(w2_sb[:], w2_d[:])
            nc.sync.dma_start(b2_sb[:], b2_d[:])

            for t0, nt in _token_chunks(C):
                ht = hpool.tile([P, FO, CHUNK], fp16, name="ht")
                for fo in range(FO):
                    ps = pspool.tile([P, CHUNK], fp32, name="ps")
                    for ko in range(KO):
                        nc.tensor.matmul(
                            ps[:, :nt],
                            lhsT=w1_sb[:, ko, fo * P:(fo + 1) * P],
                            rhs=xt_sb[:, ko, t0:t0 + nt],
                            start=(ko == 0), stop=(ko == KO - 1),
                        )
                    nc.scalar.activation(
                        ht[:, fo, :nt], ps[:, :nt],
                        mybir.ActivationFunctionType.Relu,
                        bias=b1_sb[:, fo:fo + 1],
                    )
                yt = ypool.tile([P, KO, CHUNK], fp32, name="yt")
                for ko in range(KO):
                    ps = pspool.tile([P, CHUNK], fp32, name="ps")
                    for fo in range(FO):
                        nc.tensor.matmul(
                            ps[:, :nt],
                            lhsT=w2_sb[:, fo, ko * P:(ko + 1) * P],
                            rhs=ht[:, fo, :nt],
                            start=(fo == 0), stop=(fo == FO - 1),
                        )
                    nc.scalar.activation(
                        yt[:, ko, :nt], ps[:, :nt],
                        mybir.ActivationFunctionType.Identity,
                        bias=b2_sb[:, ko:ko + 1],
                    )
                nc.sync.dma_start(yt_d[:, :, t0:t0 + nt], yt[:, :, :nt])

    nc.compile()
    _program_cache[C] = nc
    return nc


def _route(xf, Wr):
    """Host router: top-2 expert ids + softmax weights (matches lax.top_k)."""
    T = xf.shape[0]
    logits = xf @ Wr
    i1 = np.argmax(logits, axis=1)
    l1 = logits[np.arange(T), i1]
    masked = logits.copy()
    masked[np.arange(T), i1] = -np.inf
    i2 = np.argmax(masked, axis=1)
    l2 = logits[np.arange(T), i2]
    e2 = np.exp((l2 - l1).astype(np.float32))
    wt1 = 1.0 / (1.0 + e2)
    wt2 = e2 / (1.0 + e2)
    return i1, i2, wt1, wt2


def _forward(inputs, trace=False, trace_kwargs=None):
    x = np.ascontiguousarray(np.asarray(inputs["x"], dtype=np.float32))
    Wr = np.asarray(inputs["Wr"], dtype=np.float32)
    W1 = np.asarray(inputs["W1"], dtype=np.float32)
    b1 = np.asarray(inputs["b1"], dtype=np.float32)
    W2 = np.asarray(inputs["W2"], dtype=np.float32)
    b2 = np.asarray(inputs["b2"], dtype=np.float32)

    B, S, D = x.shape
    T = B * S
    xf = x.reshape(T, D)

    i1, i2, wt1, wt2 = _route(xf, Wr)
    idx = [np.nonzero((i1 == e) | (i2 == e))[0] for e in range(N_EXPERTS)]
    gw = [np.where(i1[ix] == e, wt1[ix], wt2[ix]).astype(np.float32)
          for e, ix in enumerate(idx)]

    C = max(((max(len(ix) for ix in idx) + P - 1) // P) * P, P)
    nc = _build_program(C)

    in_maps = []
    for e in range(N_EXPERTS):
        ix = idx[e]
        xe = np.zeros((C, D), dtype=np.float16)
        xe[:len(ix)] = xf[ix]
        # XT[d,t] -> [p, ko, t] with d = ko*P + p
        xt = np.ascontiguousarray(xe.T.reshape(KO, P, C).transpose(1, 0, 2))
        w1 = np.ascontiguousarray(
            W1[e].astype(np.float16).reshape(KO, P, D_FF).transpose(1, 0, 2))
        w2 = np.ascontiguousarray(
            W2[e].astype(np.float16).reshape(FO, P, D_MODEL).transpose(1, 0, 2))
        b1c = np.ascontiguousarray(b1[e].reshape(FO, P).T)
        b2c = np.ascontiguousarray(b2[e].reshape(KO, P).T)
        in_maps.append({"xt": xt, "w1": w1, "w2": w2, "b1c": b1c, "b2c": b2c})

    res = bass_utils.run_bass_kernel_spmd(
        nc, in_maps, core_ids=list(range(N_EXPERTS)), trace=trace,
        **(trace_kwargs or {}),
    )

    out = np.zeros((T, D), dtype=np.float32)
    for e in range(N_EXPERTS):
        ix = idx[e]
        if len(ix) == 0:
            continue
        # yt [p, ko, t] -> Y [t, d]
        yt = res.results[e]["yt"]
        ye = yt.transpose(2, 1, 0).reshape(C, D)[:len(ix)]
        out[ix] += gw[e][:, None] * ye
    return out.reshape(B, S, D), res


def kernel(**inputs) -> np.ndarray:
    out, _ = _forward(inputs)
    return out


# revision 2
# speedup vs baseline: 1.1221x; 1.1221x over previous
"""MoE (top-2 of 8 experts, D=768, FF=3072) on 8 Trainium2 NeuronCores.

Strategy: expert-parallel. The router (0.05 GFLOP) runs on host; tokens are
dispatched to their top-2 experts on host, each core runs one expert's FFN
over its routed tokens (the 77 GFLOP that matter), and the host applies the
softmax-weighted combine.

Device layout puts tokens on the matmul free axis, so both matmuls contract
naturally over the partition axis with zero on-device transposes:
    HT[f,t] = relu(sum_d W1[d,f] * XT[d,t] + b1[f])   lhsT=W1, rhs=XT
    YT[d,t] =      sum_f W2[f,d] * HT[f,t] + b2[d]    lhsT=W2, rhs=HT
Inputs are fp16 (well-scaled data; PSUM accumulates fp32), epilogues fp32.
Weights and tokens stream in as slices so the first matmul starts ~5us in.
"""

import numpy as np

import concourse.bass as bass
import concourse.tile as tile
from concourse import bacc, mybir
from concourse import bass_utils

D_MODEL = 768
N_EXPERTS = 8
TOP_K = 2
D_FF = 3072
P = 128
KO = D_MODEL // P     # 6   contraction chunks for MM1 / output tiles for MM2
FO = D_FF // P        # 24  output tiles for MM1 / contraction chunks for MM2
W_PARTS = 4           # weight DMA split: 4 slices of 6 f-tiles each
FO_PER_PART = FO // W_PARTS

_program_cache: dict[tuple, object] = {}


def _token_chunks(C):
    """Equal-ish chunks (multiples of 4, <=512) covering C tokens."""
    nchunks = -(-C // 512)
    base = -(-C // nchunks)
    base = -(-base // 4) * 4
    chunks = []
    t = 0
    while t < C:
        n = min(base, C - t)
        chunks.append((t, n))
        t += n
    return chunks


def _build_program(C):
    """Bass program for one expert's FFN over C routed tokens (SPMD x8)."""
    key = C
    if key in _program_cache:
        return _program_cache[key]

    fp16 = mybir.dt.float16
    fp32 = mybir.dt.float32
    nc = bacc.Bacc("TRN2", target_bir_lowering=False, debug=False,
                   enable_asserts=True, num_devices=N_EXPERTS)

    chunks = _token_chunks(C)
    cmax = max(n for _, n in chunks)

    # DRAM inputs, pre-sliced host-side so every DMA is contiguous per row
    xt_d = [nc.dram_tensor(f"xt{ci}", [P, KO, n], fp16, kind="ExternalInput").ap()
            for ci, (_, n) in enumerate(chunks)]
    w1_d = [nc.dram_tensor(f"w1_{s}", [P, KO, FO_PER_PART * P], fp16,
                           kind="ExternalInput").ap() for s in range(W_PARTS)]
    w2_d = [nc.dram_tensor(f"w2_{s}", [P, FO_PER_PART, D_MODEL], fp16,
                           kind="ExternalInput").ap() for s in range(W_PARTS)]
    b1_d = nc.dram_tensor("b1c", [P, FO], fp32, kind="ExternalInput").ap()
    b2_d = nc.dram_tensor("b2c", [P, KO], fp32, kind="ExternalInput").ap()
    yt_d = nc.dram_tensor("yt", [P, KO, C], fp32, kind="ExternalOutput").ap()

    with tile.TileContext(nc) as tc:
        with (
            tc.tile_pool(name="wpool", bufs=1) as wpool,
            tc.tile_pool(name="hpool", bufs=2) as hpool,
            tc.tile_pool(name="ypool", bufs=2) as ypool,
            tc.tile_pool(name="pspool", bufs=4, space="PSUM") as pspool,
        ):
            xt_sb = [wpool.tile([P, KO, n], fp16, name=f"xt_sb{ci}")
                     for ci, (_, n) in enumerate(chunks)]
            w1_sb = [wpool.tile([P, KO, FO_PER_PART * P], fp16, name=f"w1_sb{s}")
                     for s in range(W_PARTS)]
            w2_sb = [wpool.tile([P, FO_PER_PART, D_MODEL], fp16, name=f"w2_sb{s}")
                     for s in range(W_PARTS)]
            b1_sb = wpool.tile([P, FO], fp32)
            b2_sb = wpool.tile([P, KO], fp32)

            # DMA order = need order: first chunk tokens + first w1 slice
            # unblock the first matmuls; w2 streams in under chunk-0 MM1.
            nc.sync.dma_start(xt_sb[0][:], xt_d[0][:])
            nc.sync.dma_start(w1_sb[0][:], w1_d[0][:])
            nc.sync.dma_start(b1_sb[:], b1_d[:])
            for s in range(1, W_PARTS):
                nc.sync.dma_start(w1_sb[s][:], w1_d[s][:])
            for ci in range(1, len(chunks)):
                nc.sync.dma_start(xt_sb[ci][:], xt_d[ci][:])
            for s in range(W_PARTS):
                nc.sync.dma_start(w2_sb[s][:], w2_d[s][:])
            nc.sync.dma_start(b2_sb[:], b2_d[:])

            for ci, (t0, nt) in enumerate(chunks):
                ht = hpool.tile([P, FO, cmax], fp16, name="ht")
                for fo in range(FO):
                    s, f = divmod(fo, FO_PER_PART)
                    ps = pspool.tile([P, cmax], fp32, name="ps")
                    for ko in range(KO):
                        nc.tensor.matmul(
                            ps[:, :nt],
                            lhsT=w1_sb[s][:, ko, f * P:(f + 1) * P],
                            rhs=xt_sb[ci][:, ko, :nt],
                            start=(ko == 0), stop=(ko == KO - 1),
                        )
                    nc.scalar.activation(
                        ht[:, fo, :nt], ps[:, :nt],
                        mybir.ActivationFunctionType.Relu,
                        bias=b1_sb[:, fo:fo + 1],
                    )
                yt = ypool.tile([P, KO, cmax], fp32, name="yt")
                for ko in range(KO):
                    ps = pspool.tile([P, cmax], fp32, name="ps")
                    for fo in range(FO):
                        s, f = divmod(fo, FO_PER_PART)
                        nc.tensor.matmul(
                            ps[:, :nt],
                            lhsT=w2_sb[s][:, f, ko * P:(ko + 1) * P],
                            rhs=ht[:, fo, :nt],
                            start=(fo == 0), stop=(fo == FO - 1),
                        )
                    nc.scalar.activation(
                        yt[:, ko, :nt], ps[:, :nt],
                        mybir.ActivationFunctionType.Identity,
                        bias=b2_sb[:, ko:ko + 1],
                    )
                    nc.sync.dma_start(yt_d[:, ko, t0:t0 + nt], yt[:, ko, :nt])

    nc.compile()
    _program_cache[key] = nc
    return nc


def _route(xf, Wr):
    """Host router: top-2 expert ids + softmax weights (matches lax.top_k)."""
    T = xf.shape[0]
    logits = xf @ Wr
    i1 = np.argmax(logits, axis=1)
    l1 = logits[np.arange(T), i1]
    masked = logits.copy()
    masked[np.arange(T), i1] = -np.inf
    i2 = np.argmax(masked, axis=1)
    l2 = logits[np.arange(T), i2]
    e2 = np.exp((l2 - l1).astype(np.float32))
    wt1 = 1.0 / (1.0 + e2)
    wt2 = e2 / (1.0 + e2)
    return i1, i2, wt1, wt2


def _forward(inputs, trace=False, trace_kwargs=None):
    x = np.ascontiguousarray(np.asarray(inputs["x"], dtype=np.float32))
    Wr = np.asarray(inputs["Wr"], dtype=np.float32)
    W1 = np.asarray(inputs["W1"], dtype=np.float32)
    b1 = np.asarray(inputs["b1"], dtype=np.float32)
    W2 = np.asarray(inputs["W2"], dtype=np.float32)
    b2 = np.asarray(inputs["b2"], dtype=np.float32)

    B, S, D = x.shape
    T = B * S
    xf = x.reshape(T, D)

    i1, i2, wt1, wt2 = _route(xf, Wr)
    idx = [np.nonzero((i1 == e) | (i2 == e))[0] for e in range(N_EXPERTS)]
    gw = [np.where(i1[ix] == e, wt1[ix], wt2[ix]).astype(np.float32)
          for e, ix in enumerate(idx)]

    C = max(-(-max(len(ix) for ix in idx) // 4) * 4, 4)
    nc = _build_program(C)
    chunks = _token_chunks(C)

    in_maps = []
    for e in range(N_EXPERTS):
        ix = idx[e]
        xe = np.zeros((C, D), dtype=np.float16)
        xe[:len(ix)] = xf[ix]
        # XT[d,t] -> [p, ko, t] with d = ko*P + p
        xt = np.ascontiguousarray(xe.T.reshape(KO, P, C).transpose(1, 0, 2))
        w1 = np.ascontiguousarray(
            W1[e].astype(np.float16).reshape(KO, P, D_FF).transpose(1, 0, 2))
        w2 = np.ascontiguousarray(
            W2[e].astype(np.float16).reshape(FO, P, D_MODEL).transpose(1, 0, 2))
        m = {"b1c": np.ascontiguousarray(b1[e].reshape(FO, P).T),
             "b2c": np.ascontiguousarray(b2[e].reshape(KO, P).T)}
        for ci, (t0, n) in enumerate(chunks):
            m[f"xt{ci}"] = np.ascontiguousarray(xt[:, :, t0:t0 + n])
        for s in range(W_PARTS):
            f0 = s * FO_PER_PART * P
            m[f"w1_{s}"] = np.ascontiguousarray(w1[:, :, f0:f0 + FO_PER_PART * P])
            m[f"w2_{s}"] = np.ascontiguousarray(
                w2[:, s * FO_PER_PART:(s + 1) * FO_PER_PART, :])
        in_maps.append(m)

    res = bass_utils.run_bass_kernel_spmd(
        nc, in_maps, core_ids=list(range(N_EXPERTS)), trace=trace,
        **(trace_kwargs or {}),
    )

    out = np.zeros((T, D), dtype=np.float32)
    for e in range(N_EXPERTS):
        ix = idx[e]
        if len(ix) == 0:
            continue
        # yt [p, ko, t] -> Y [t, d]
        yt = res.results[e]["yt"]
        ye = yt.transpose(2, 1, 0).reshape(C, D)[:len(ix)]
        out[ix] += gw[e][:, None] * ye
    return out.reshape(B, S, D), res


def kernel(**inputs) -> np.ndarray:
    out, _ = _forward(inputs)
    return out


# revision 6
# speedup vs baseline: 1.1465x; 1.0217x over previous
"""MoE (top-2 of 8 experts, D=768, FF=3072) on 8 Trainium2 NeuronCores.

Strategy: expert-parallel. The router (0.05 GFLOP) runs on host; tokens are
dispatched to their top-2 experts on host, each core runs one expert's FFN
over its routed tokens (the 77 GFLOP that matter), and the host applies the
softmax-weighted combine.

Device layout puts tokens on the matmul free axis, so both matmuls contract
naturally over the partition axis with zero on-device transposes:
    HT[f,t] = relu(sum_d W1[d,f] * XT[d,t] + b1[f])   lhsT=W1, rhs=XT
    YT[d,t] =      sum_f W2[f,d] * HT[f,t] + b2[d]    lhsT=W2, rhs=HT
Inputs are fp16 (well-scaled data; PSUM accumulates fp32), epilogues fp32.
Weights and tokens stream in as slices so the first matmul starts ~5us in.
"""

import numpy as np

import concourse.bass as bass
import concourse.tile as tile
from concourse import bacc, mybir
from concourse import bass_utils

D_MODEL = 768
N_EXPERTS = 8
TOP_K = 2
D_FF = 3072
P = 128
KO = D_MODEL // P     # 6   contraction chunks for MM1 / output tiles for MM2
FO = D_FF // P        # 24  output tiles for MM1 / contraction chunks for MM2
W1_PARTS = 8          # w1 DMA split: 8 slices of 3 f-tiles (finer => MM1 starts sooner)
FO_PER_W1 = FO // W1_PARTS
W_PARTS = 4           # w2 DMA split: 4 slices of 6 f-tiles each
FO_PER_PART = FO // W_PARTS
WARMUP_MMS = 20       # dummy matmuls during the DMA prologue keep HAM at 2.4GHz

_program_cache: dict[tuple, object] = {}


def _token_chunks(C):
    """Equal-ish chunks (multiples of 4, <=512) covering C tokens."""
    nchunks = -(-C // 512)
    base = -(-C // nchunks)
    base = -(-base // 4) * 4
    chunks = []
    t = 0
    while t < C:
        n = min(base, C - t)
        chunks.append((t, n))
        t += n
    return chunks


def _build_program(C):
    """Bass program for one expert's FFN over C routed tokens (SPMD x8)."""
    key = C
    if key in _program_cache:
        return _program_cache[key]

    fp16 = mybir.dt.float16
    fp32 = mybir.dt.float32
    nc = bacc.Bacc("TRN2", target_bir_lowering=False, debug=False,
                   enable_asserts=True, num_devices=N_EXPERTS)

    chunks = _token_chunks(C)
    cmax = max(n for _, n in chunks)

    # DRAM inputs, pre-sliced host-side so every DMA is contiguous per row
    xt_d = [nc.dram_tensor(f"xt{ci}", [P, KO, n], fp16, kind="ExternalInput").ap()
            for ci, (_, n) in enumerate(chunks)]
    w1_d = [nc.dram_tensor(f"w1_{s}", [P, KO, FO_PER_W1 * P], fp16,
                           kind="ExternalInput").ap() for s in range(W1_PARTS)]
    w2_d = [nc.dram_tensor(f"w2_{s}", [P, FO_PER_PART, D_MODEL], fp16,
                           kind="ExternalInput").ap() for s in range(W_PARTS)]
    b1_d = nc.dram_tensor("b1c", [P, FO], fp32, kind="ExternalInput").ap()
    b2_d = nc.dram_tensor("b2c", [P, KO], fp32, kind="ExternalInput").ap()
    yt_d = nc.dram_tensor("yt", [P, KO, C], fp32, kind="ExternalOutput").ap()

    with tile.TileContext(nc) as tc:
        with (
            tc.tile_pool(name="wpool", bufs=1) as wpool,
            tc.tile_pool(name="hpool", bufs=2) as hpool,
            tc.tile_pool(name="ypool", bufs=2) as ypool,
            tc.tile_pool(name="pspool", bufs=4, space="PSUM") as pspool,
        ):
            xt_sb = [wpool.tile([P, KO, n], fp16, name=f"xt_sb{ci}")
                     for ci, (_, n) in enumerate(chunks)]
            w1_sb = [wpool.tile([P, KO, FO_PER_W1 * P], fp16, name=f"w1_sb{s}")
                     for s in range(W1_PARTS)]
            w2_sb = [wpool.tile([P, FO_PER_PART, D_MODEL], fp16, name=f"w2_sb{s}")
                     for s in range(W_PARTS)]
            b1_sb = wpool.tile([P, FO], fp32)
            b2_sb = wpool.tile([P, KO], fp32)

            # PE warmup: dummy matmuls on a zeroed tile fill the DMA
            # prologue so the HAM clock-gate reaches 2.4GHz before the
            # real matmuls start.
            warm = wpool.tile([P, 512], fp16)
            nc.gpsimd.memset(warm[:], 0.0)
            ps_w = pspool.tile([P, 512], fp32, name="ps_w", bufs=1)
            for _ in range(WARMUP_MMS):
                nc.tensor.matmul(ps_w[:], lhsT=warm[:, :P], rhs=warm[:],
                                 start=True, stop=True)

            # DMA order = need order: first chunk tokens + first w1 slice
            # unblock the first matmuls; w2 streams in under chunk-0 MM1.
            nc.sync.dma_start(xt_sb[0][:], xt_d[0][:])
            nc.sync.dma_start(w1_sb[0][:], w1_d[0][:])
            nc.sync.dma_start(b1_sb[:], b1_d[:])
            for s in range(1, W1_PARTS):
                nc.sync.dma_start(w1_sb[s][:], w1_d[s][:])
            for ci in range(1, len(chunks)):
                nc.sync.dma_start(xt_sb[ci][:], xt_d[ci][:])
            for s in range(W_PARTS):
                nc.sync.dma_start(w2_sb[s][:], w2_d[s][:])
            nc.sync.dma_start(b2_sb[:], b2_d[:])

            for ci, (t0, nt) in enumerate(chunks):
                ht = hpool.tile([P, FO, cmax], fp16, name="ht")
                for fo in range(FO):
                    s, f = divmod(fo, FO_PER_W1)
                    ps = pspool.tile([P, cmax], fp32, name="ps")
                    for ko in range(KO):
                        nc.tensor.matmul(
                            ps[:, :nt],
                            lhsT=w1_sb[s][:, ko, f * P:(f + 1) * P],
                            rhs=xt_sb[ci][:, ko, :nt],
                            start=(ko == 0), stop=(ko == KO - 1),
                        )
                    nc.scalar.activation(
                        ht[:, fo, :nt], ps[:, :nt],
                        mybir.ActivationFunctionType.Relu,
                        bias=b1_sb[:, fo:fo + 1],
                    )
                yt = ypool.tile([P, KO, cmax], fp32, name="yt")
                for ko in range(KO):
                    ps = pspool.tile([P, cmax], fp32, name="ps")
                    for fo in range(FO):
                        s, f = divmod(fo, FO_PER_PART)
                        nc.tensor.matmul(
                            ps[:, :nt],
                            lhsT=w2_sb[s][:, f, ko * P:(ko + 1) * P],
                            rhs=ht[:, fo, :nt],
                            start=(fo == 0), stop=(fo == FO - 1),
                        )
                    nc.scalar.activation(
                        yt[:, ko, :nt], ps[:, :nt],
                        mybir.ActivationFunctionType.Identity,
                        bias=b2_sb[:, ko:ko + 1],
                    )
                    nc.sync.dma_start(yt_d[:, ko, t0:t0 + nt], yt[:, ko, :nt])

    nc.compile()
    _program_cache[key] = nc
    return nc


def _route(xf, Wr):
    """Host router: top-2 expert ids + softmax weights (matches lax.top_k)."""
    T = xf.shape[0]
    logits = xf @ Wr
    i1 = np.argmax(logits, axis=1)
    l1 = logits[np.arange(T), i1]
    masked = logits.copy()
    masked[np.arange(T), i1] = -np.inf
    i2 = np.argmax(masked, axis=1)
    l2 = logits[np.arange(T), i2]
    e2 = np.exp((l2 - l1).astype(np.float32))
    wt1 = 1.0 / (1.0 + e2)
    wt2 = e2 / (1.0 + e2)
    return i1, i2, wt1, wt2


def _forward(inputs, trace=False, trace_kwargs=None):
    x = np.ascontiguousarray(np.asarray(inputs["x"], dtype=np.float32))
    Wr = np.asarray(inputs["Wr"], dtype=np.float32)
    W1 = np.asarray(inputs["W1"], dtype=np.float32)
    b1 = np.asarray(inputs["b1"], dtype=np.float32)
    W2 = np.asarray(inputs["W2"], dtype=np.float32)
    b2 = np.asarray(inputs["b2"], dtype=np.float32)

    B, S, D = x.shape
    T = B * S
    xf = x.reshape(T, D)

    i1, i2, wt1, wt2 = _route(xf, Wr)
    idx = [np.nonzero((i1 == e) | (i2 == e))[0] for e in range(N_EXPERTS)]
    gw = [np.where(i1[ix] == e, wt1[ix], wt2[ix]).astype(np.float32)
          for e, ix in enumerate(idx)]

    C = max(-(-max(len(ix) for ix in idx) // 4) * 4, 4)
    nc = _build_program(C)
    chunks = _token_chunks(C)

    in_maps = []
    for e in range(N_EXPERTS):
        ix = idx[e]
        xe = np.zeros((C, D), dtype=np.float16)
        xe[:len(ix)] = xf[ix]
        # XT[d,t] -> [p, ko, t] with d = ko*P + p
        xt = np.ascontiguousarray(xe.T.reshape(KO, P, C).transpose(1, 0, 2))
        w1 = np.ascontiguousarray(
            W1[e].astype(np.float16).reshape(KO, P, D_FF).transpose(1, 0, 2))
        w2 = np.ascontiguousarray(
            W2[e].astype(np.float16).reshape(FO, P, D_MODEL).transpose(1, 0, 2))
        m = {"b1c": np.ascontiguousarray(b1[e].reshape(FO, P).T),
             "b2c": np.ascontiguousarray(b2[e].reshape(KO, P).T)}
        for ci, (t0, n) in enumerate(chunks):
            m[f"xt{ci}"] = np.ascontiguousarray(xt[:, :, t0:t0 + n])
        for s in range(W1_PARTS):
            f0 = s * FO_PER_W1 * P
            m[f"w1_{s}"] = np.ascontiguousarray(w1[:, :, f0:f0 + FO_PER_W1 * P])
        for s in range(W_PARTS):
            m[f"w2_{s}"] = np.ascontiguousarray(
                w2[:, s * FO_PER_PART:(s + 1) * FO_PER_PART, :])
        in_maps.append(m)

    res = bass_utils.run_bass_kernel_spmd(
        nc, in_maps, core_ids=list(range(N_EXPERTS)), trace=trace,
        **(trace_kwargs or {}),
    )

    out = np.zeros((T, D), dtype=np.float32)
    for e in range(N_EXPERTS):
        ix = idx[e]
        if len(ix) == 0:
            continue
        # yt [p, ko, t] -> Y [t, d]
        yt = res.results[e]["yt"]
        ye = yt.transpose(2, 1, 0).reshape(C, D)[:len(ix)]
        out[ix] += gw[e][:, None] * ye
    return out.reshape(B, S, D), res


def kernel(**inputs) -> np.ndarray:
    out, _ = _forward(inputs)
    return out
